# revision 12
# baseline (speedup 1.0000x reference)
"""CrossViewTransformer Trainium2 kernel (v2).

Shards batch B=4 x row-halves over 8 NeuronCores (pure data parallel,
one program, per-core data). Per core:
  q = Wq @ cross_ext          (32, 2176)   fp16 hi/lo split MMs (exact-ish)
  k = Wk @ front_x            (32, 4096)
  energy[j,i] = <q_j, k_i>    K=128 fp16 [qh;ql;qh;ql]x[kh;kh;kl;kl] MMs
  argmax/max via DVE running-max scan + is_lt count (first-occurrence exact)
  v = Wv @ x_hat              fp16 MM, gathered by argmax via gpsimd ap_gather
  conv3x3([front_x; T]) * S + front_x   fp16 MMs; front half overlapped
                                        with the energy loop, T half after.

All weights are pre-transposed/pre-split to fp16 on the host; activations
are pre-split fp16 hi/lo on the host. No PE transposes, no gpsimd iota.
"""
import sys

sys.path.insert(0, "/opt/trn_rl_repo")
import numpy as np  # noqa: E402
import concourse.bacc as bacc  # noqa: E402
import concourse.mybir as mybir  # noqa: E402
import concourse.tile as tile  # noqa: E402
from concourse import bass_utils  # noqa: E402
from concourse.bass import AP  # noqa: E402

dt = mybir.dt
ALU = mybir.AluOpType
AX = mybir.AxisListType

B, C, H, W = 4, 256, 64, 64
C8 = C // 8            # 32
HWF = H * W            # 4096 keys
RH = H // 2            # 32 out rows per core
EXTR = RH + 2          # 34 ext rows (1 halo/zero row each side)
EXTQ = EXTR * W        # 2176 ext queries
NBLK = EXTQ // 128     # 17 query blocks
OUTP = RH * W          # 2048 out positions
WP = W + 2             # 66 padded width
CATW = EXTR * WP       # 2244 padded cat row-major size
NCHUNK = 4             # energy chunks of 1024 keys
VCOLS = HWF + 4        # v buffer cols (idx HWF -> zero column), 4-aligned

_CACHED = {}


def _build(has_bqk: bool, has_bv: bool):
    key = (has_bqk, has_bv)
    if key in _CACHED:
        return _CACHED[key]
    nc = bacc.Bacc("TRN2", debug=False)

    # fp16 pre-split activations
    cxh_d = nc.dram_tensor("cxh", (2, 128, EXTQ), dt.float16, kind="ExternalInput")
    cxl_d = nc.dram_tensor("cxl", (2, 128, EXTQ), dt.float16, kind="ExternalInput")
    fxh_d = nc.dram_tensor("fxh", (2, 128, HWF), dt.float16, kind="ExternalInput")
    fxl_d = nc.dram_tensor("fxl", (2, 128, HWF), dt.float16, kind="ExternalInput")
    xh16_d = nc.dram_tensor("xh16", (2, 128, HWF), dt.float16, kind="ExternalInput")
    catf_d = nc.dram_tensor("catf", (2, 128, CATW), dt.float16, kind="ExternalInput")
    # fp16 pre-transposed weights
    wqkT_d = nc.dram_tensor("wqkT", (128, 8 * C8), dt.float16, kind="ExternalInput")
    wvT_d = nc.dram_tensor("wvT", (128, 4 * 128), dt.float16, kind="ExternalInput")
    wfT_d = nc.dram_tensor("wfT", (128, 72 * 128), dt.float16, kind="ExternalInput")
    bq_d = nc.dram_tensor("bq", (C8, 1), dt.float32, kind="ExternalInput")
    bk_d = nc.dram_tensor("bk", (C8, 1), dt.float32, kind="ExternalInput")
    bv_d = nc.dram_tensor("bv", (128, 2), dt.float32, kind="ExternalInput")
    bf_d = nc.dram_tensor("bf", (128, 2), dt.float32, kind="ExternalInput")
    mask_d = nc.dram_tensor("mask", (128, NBLK), dt.float32, kind="ExternalInput")
    amask_d = nc.dram_tensor("amask", (128, NBLK), dt.float32, kind="ExternalInput")

    out_d = nc.dram_tensor("out", (2, 128, OUTP), dt.float32, kind="ExternalOutput")
    dbg_arg_d = nc.dram_tensor("dbg_arg", (128, NBLK), dt.float32, kind="ExternalOutput")
    dbg_s_d = nc.dram_tensor("dbg_s", (128, NBLK), dt.float32, kind="ExternalOutput")

    with tile.TileContext(nc) as tc:
        _body(nc, tc, locals(), has_bqk, has_bv)
    nc.compile()
    _CACHED[key] = nc
    return nc


def _body(nc, tc, T, has_bqk, has_bv):
    F32, F16, I16 = dt.float32, dt.float16, dt.int16

    with tc.tile_pool(name="dramscr", bufs=1, space="DRAM") as DR, \
         tc.tile_pool(name="persist", bufs=1) as P, \
         tc.tile_pool(name="stream", bufs=2) as S, \
         tc.tile_pool(name="pse", bufs=3, space="PSUM") as PSE, \
         tc.tile_pool(name="psb", bufs=2, space="PSUM") as PSB:

        # ---------- persistent tiles ----------
        wqkT = P.tile([128, 8 * C8], F16, tag="wqkT")
        wvT = P.tile([128, 4 * 128], F16, tag="wvT")
        wfT = P.tile([128, 72 * 128], F16, tag="wfT")
        qstack = P.tile([128, EXTQ], F16, tag="qstack")
        kstack = P.tile([128, HWF], F16, tag="kstack")
        vbuf0 = P.tile([128, VCOLS], F32, tag="vbuf0")
        vbuf1 = P.tile([128, VCOLS], F32, tag="vbuf1")
        vbufs = (vbuf0, vbuf1)
        rbuf = P.tile([128, HWF], F32, tag="rbuf")
        scr = P.tile([128, HWF], F16, tag="scr")
        cats = []
        for i in range(4):
            ct = P.tile([128, CATW], F16, tag=f"cat{i}")
            cats.append(ct)
        convacc = P.tile([128, 8, 512], F32, tag="convacc")  # (ob*4+g)
        s128 = P.tile([128, OUTP], F32, tag="s128")
        SM = P.tile([128, 128], F32, tag="smalls")
        Af = SM[:, 0:NBLK]
        Mg = SM[:, 17:17 + NBLK]
        arg2 = SM[:, 34:34 + NBLK]
        maskt = SM[:, 51:51 + NBLK]
        amaskt = SM[:, 68:68 + NBLK]
        bqs = SM[0:C8, 85:87]
        bvs = SM[:, 87:89]
        bfs = SM[:, 89:91]
        af16 = P.tile([128, NBLK], I16, tag="af16")
        idxw = P.tile([128, EXTQ // 16], I16, tag="idxw")

        # ---------- input DMAs ----------
        nc.sync.dma_start(wqkT[:, :], T["wqkT_d"].ap())

        ACT_cm = tc.tile_pool(name="actpool", bufs=1)
        ACT = ACT_cm.__enter__()
        cxh = ACT.tile([128, 2, EXTQ], F16, tag="cxh")
        cxl = ACT.tile([128, 2, EXTQ], F16, tag="cxl")
        fxh = ACT.tile([128, 2, HWF], F16, tag="fxh")
        fxl = ACT.tile([128, 2, HWF], F16, tag="fxl")
        xh16 = ACT.tile([128, 2, HWF], F16, tag="xh16")
        for cb in range(2):
            nc.sync.dma_start(cxh[:, cb], T["cxh_d"].ap()[cb])
            nc.sync.dma_start(cxl[:, cb], T["cxl_d"].ap()[cb])
        for cb in range(2):
            nc.sync.dma_start(fxh[:, cb], T["fxh_d"].ap()[cb])
            nc.sync.dma_start(fxl[:, cb], T["fxl_d"].ap()[cb])
        nc.sync.dma_start(wvT[:, :], T["wvT_d"].ap())
        for cb in range(2):
            nc.sync.dma_start(xh16[:, cb], T["xh16_d"].ap()[cb])
        nc.sync.dma_start(wfT[:, :], T["wfT_d"].ap())
        for cb in range(2):
            nc.sync.dma_start(cats[cb][:, :], T["catf_d"].ap()[cb])
        nc.sync.dma_start(maskt[:, :], T["mask_d"].ap())
        nc.sync.dma_start(amaskt[:, :], T["amask_d"].ap())
        if has_bqk:
            nc.sync.dma_start(bqs[:, 0:1], T["bq_d"].ap())
            nc.sync.dma_start(bqs[:, 1:2], T["bk_d"].ap())
        if has_bv:
            nc.sync.dma_start(bvs[:, :], T["bv_d"].ap())
        nc.sync.dma_start(bfs[:, :], T["bf_d"].ap())

        nc.vector.memset(cats[2][:, :], 0.0)
        nc.vector.memset(cats[3][:, :], 0.0)
        nc.vector.memset(vbuf0[:, HWF:VCOLS], 0.0)
        nc.vector.memset(vbuf1[:, HWF:VCOLS], 0.0)

        # ---------- q, k (fp16 hi/lo x hi/lo accumulation) ----------
        def qk_mm(which, xh_t, xl_t, npos, stack, hrows, lrows):
            nchunks = (npos + 511) // 512
            for ch in range(nchunks):
                n0, n1 = ch * 512, min((ch + 1) * 512, npos)
                pq = PSB.tile([C8, 512], F32, tag="ps512")
                first = True
                for cb in range(2):
                    for wsplit in range(2):
                        wcol = ((which * 2 + cb) * 2 + wsplit) * C8
                        for xs, xt in ((0, xh_t), (1, xl_t)):
                            nc.tensor.matmul(
                                pq[:, 0:n1 - n0],
                                wqkT[:, wcol:wcol + C8],
                                xt[:, cb, n0:n1],
                                start=first, stop=(cb == 1 and wsplit == 1 and xs == 1))
                            first = False
                hi = stack[hrows[0]:hrows[0] + C8, n0:n1]
                if has_bqk:
                    nc.vector.tensor_scalar(
                        out=hi, in0=pq[:, 0:n1 - n0],
                        scalar1=bqs[:, which:which + 1], scalar2=None, op0=ALU.add)
                else:
                    nc.scalar.copy(hi, pq[:, 0:n1 - n0])
                nc.vector.scalar_tensor_tensor(
                    stack[lrows[0]:lrows[0] + C8, n0:n1], pq[:, 0:n1 - n0],
                    bqs[:, which:which + 1] if has_bqk else 0.0, hi,
                    op0=ALU.add, op1=ALU.subtract)
                for extra in hrows[1:]:
                    nc.vector.tensor_copy(stack[extra:extra + C8, n0:n1], hi)
                for extra in lrows[1:]:
                    nc.vector.tensor_copy(stack[extra:extra + C8, n0:n1],
                                          stack[lrows[0]:lrows[0] + C8, n0:n1])

        # qstack rows: [qh, ql, qh, ql] ; kstack rows: [kh, kh, kl, kl]
        qk_mm(0, cxh, cxl, EXTQ, qstack, hrows=(0, 64), lrows=(32, 96))
        qk_mm(1, fxh, fxl, HWF, kstack, hrows=(0, 32), lrows=(64, 96))

        # ---------- filler emitters (PE work overlapped with energy DVE) ----------
        def emit_v_chunk(ch):
            for ob in range(2):
                pv = PSB.tile([128, 512], F32, tag="ps512")
                for cb in range(2):
                    nc.tensor.matmul(pv[:, :],
                                     wvT[:, (cb * 2 + ob) * 128:(cb * 2 + ob + 1) * 128],
                                     xh16[:, cb, ch * 512:(ch + 1) * 512],
                                     start=(cb == 0), stop=(cb == 1))
                if has_bv:
                    nc.vector.tensor_scalar(
                        out=vbufs[ob][:, ch * 512:(ch + 1) * 512], in0=pv[:, :],
                        scalar1=bvs[:, ob:ob + 1], scalar2=None, op0=ALU.add)
                else:
                    nc.scalar.copy(vbufs[ob][:, ch * 512:(ch + 1) * 512], pv[:, :])

        def emit_conv_half(ob, g, cb4s, pc):
            # accumulate taps of cb4s into pc ([128,512])
            for cb4 in cb4s:
                for tap in range(9):
                    dy, dx = tap // 3, tap % 3
                    col = ((cb4 * 9 + tap) * 2 + ob) * 128
                    catv = cats[cb4][:, :].rearrange("p (r wp) -> p r wp", wp=WP)
                    rhs = catv[:, g * 8 + dy:g * 8 + dy + 8, dx:dx + W]
                    nc.tensor.matmul(pc[:, :], wfT[:, col:col + 128], rhs,
                                     start=(cb4 == cb4s[0] and tap == 0),
                                     stop=(cb4 == cb4s[-1] and tap == 8))

        fillers = []
        for ch in range(8):
            fillers.append(lambda ch=ch: emit_v_chunk(ch))

        def emit_front_group(ob, g):
            pc = PSB.tile([128, 512], F32, tag="ps512")
            emit_conv_half(ob, g, (0, 1), pc)
            nc.scalar.copy(convacc[:, ob * 4 + g, :], pc[:, :])

        for ob in range(2):
            for g in range(4):
                fillers.append(lambda ob=ob, g=g: emit_front_group(ob, g))

        # ---------- energy + argmax (running-max scan + count) ----------
        for b in range(NBLK):
            for c in range(NCHUNK):
                pe = PSE.tile([128, 1024], F32, tag="pe")
                nc.tensor.matmul(pe[:, 0:512], qstack[:, b * 128:(b + 1) * 128],
                                 kstack[:, c * 1024:c * 1024 + 512], start=True, stop=True)
                nc.tensor.matmul(pe[:, 512:1024], qstack[:, b * 128:(b + 1) * 128],
                                 kstack[:, c * 1024 + 512:(c + 1) * 1024], start=True, stop=True)
                init = -1e30 if c == 0 else rbuf[:, c * 1024 - 1:c * 1024]
                nc.vector.tensor_tensor_scan(
                    rbuf[:, c * 1024:(c + 1) * 1024], pe[:, :],
                    kstack[:, c * 1024:(c + 1) * 1024], init,
                    op0=ALU.max, op1=ALU.bypass)
            # one PE filler batch per block
            if b < len(fillers):
                fillers[b]()
            if b == 7:
                ACT_cm.__exit__(None, None, None)  # cx/fx/xh dead: free SBUF
            nc.vector.tensor_scalar(out=scr[:, :], in0=rbuf[:, :],
                                    scalar1=rbuf[:, HWF - 1:HWF], scalar2=None,
                                    op0=ALU.is_lt, op1=ALU.add,
                                    accum_out=Af[:, b:b + 1])
            nc.scalar.copy(Mg[:, b:b + 1], rbuf[:, HWF - 1:HWF])
        # any unused fillers
        for f in fillers[NBLK:]:
            f()

        nc.sync.dma_start(T["dbg_s_d"].ap(), Mg[:, :])
        nc.sync.dma_start(T["dbg_arg_d"].ap(), Af[:, :])

        # masked arg: arg2 = Af*mask + amask  (amask = (1-mask)*HWF)
        nc.vector.tensor_tensor(arg2[:, :], Af[:, :], maskt[:, :], op=ALU.mult)
        nc.vector.tensor_tensor(arg2[:, :], arg2[:, :], amaskt[:, :], op=ALU.add)
        nc.vector.tensor_copy(af16[:, :], arg2[:, :])

        # wrapped-index relayout via DRAM bounce:
        # wrap[p0*136 + b*8 + t] = af16[p=t*16+p0, b]   (query i=b*128+p at (i%16, i//16))
        wrap_t = DR.tile([EXTQ], I16, tag="wrapl")
        wsrc = wrap_t[:]
        wdst = AP(wsrc.tensor, wsrc.offset, [[1, 8], [136, 16], [8, NBLK]])
        nc.sync.dma_start(wdst, af16[:, :])
        for g in range(8):
            nc.sync.dma_start(idxw[16 * g:16 * (g + 1), :],
                              wrap_t[:].rearrange("(p0 s) -> p0 s", p0=16))

        # S row: srow[q=b*128+p] = Mg[p,b]; then stride-0 broadcast of out cols
        srow_t = DR.tile([EXTQ], F32, tag="srowd")
        ssrc = srow_t[:]
        sdst = AP(ssrc.tensor, ssrc.offset, [[1, 128], [128, NBLK]])
        nc.sync.dma_start(sdst, Mg[:, :])
        sbc = AP(ssrc.tensor, ssrc.offset + W, [[0, 128], [1, OUTP]])
        nc.sync.dma_start(s128[:, :], sbc)

        # ---------- gather T + conv T-half + assembly ----------
        TG_cm = tc.tile_pool(name="tgpool", bufs=2)
        TG = TG_cm.__enter__()
        for ob in range(2):
            tg = TG.tile([128, EXTQ], F32, tag="tg")
            nc.gpsimd.ap_gather(tg[:, :], vbufs[ob][:, 0:VCOLS], idxw[:, :],
                                channels=128, num_elems=VCOLS, d=1, num_idxs=EXTQ)
            nc.vector.tensor_copy(
                cats[2 + ob][:, :].rearrange("p (r wp) -> p r wp", wp=WP)[:, :, 1:W + 1],
                tg[:, :].rearrange("p (r w) -> p r w", w=W))

        for ob in range(2):
            for g in range(4):
                pc = PSB.tile([128, 512], F32, tag="ps512")
                emit_conv_half(ob, g, (2, 3), pc)
                stage = S.tile([128, 512], F32, tag="stage")
                nc.vector.scalar_tensor_tensor(
                    stage[:, :], pc[:, :], bfs[:, ob:ob + 1],
                    convacc[:, ob * 4 + g, :], op0=ALU.add, op1=ALU.add)
                nc.vector.tensor_tensor(stage[:, :], stage[:, :],
                                        s128[:, g * 512:(g + 1) * 512], op=ALU.mult)
                fcatv = cats[ob][:, :].rearrange("p (r wp) -> p r wp", wp=WP)
                front_mid = fcatv[:, g * 8 + 1:g * 8 + 9, 1:1 + W]
                nc.vector.tensor_tensor(stage[:, :], stage[:, :], front_mid, op=ALU.add)
                nc.sync.dma_start(T["out_d"].ap()[ob][:, g * 512:(g + 1) * 512],
                                  stage[:, :])
        TG_cm.__exit__(None, None, None)


def _prep_shared(inputs):
    """Weight prep shared by all cores: pre-transposed fp16 (+hi/lo for q/k)."""
    f16, f32 = np.float16, np.float32
    Wq, Wk, Wv = inputs["Wq"], inputs["Wk"], inputs["Wv"]
    Wf = inputs["Wf"].reshape(C, 2 * C, 9)

    wqkT = np.zeros((128, 8 * C8), f16)
    for which, Wx in ((0, Wq), (1, Wk)):
        for cb in range(2):
            blk = np.ascontiguousarray(Wx[:, cb * 128:(cb + 1) * 128].T)  # [128, 32] f32
            hi = blk.astype(f16)
            lo = (blk - hi.astype(f32)).astype(f16)
            base = ((which * 2 + cb) * 2) * C8
            wqkT[:, base:base + C8] = hi
            wqkT[:, base + C8:base + 2 * C8] = lo

    wvT = np.zeros((128, 4 * 128), f16)
    for ob in range(2):
        for cb in range(2):
            wvT[:, (cb * 2 + ob) * 128:(cb * 2 + ob + 1) * 128] = \
                Wv[ob * 128:(ob + 1) * 128, cb * 128:(cb + 1) * 128].T.astype(f16)

    wfT = np.zeros((128, 72 * 128), f16)
    for ob in range(2):
        for cb4 in range(4):
            for tap in range(9):
                col = ((cb4 * 9 + tap) * 2 + ob) * 128
                wfT[:, col:col + 128] = \
                    Wf[ob * 128:(ob + 1) * 128, cb4 * 128:(cb4 + 1) * 128, tap].T.astype(f16)

    return {
        "wqkT": wqkT, "wvT": wvT, "wfT": wfT,
        "bq": inputs["bq"].reshape(C8, 1).astype(f32),
        "bk": inputs["bk"].reshape(C8, 1).astype(f32),
        "bv": np.ascontiguousarray(inputs["bv"].reshape(2, 128).T).astype(f32),
        "bf": np.ascontiguousarray(inputs["bf"].reshape(2, 128).T).astype(f32),
    }


def _hilo(x):
    f16, f32 = np.float16, np.float32
    hi = x.astype(f16)
    lo = (x - hi.astype(f32)).astype(f16)
    return hi, lo


def _prep_core_inputs(inputs, shared, core):
    f16, f32 = np.float16, np.float32
    b, half = core // 2, core % 2
    r0 = half * RH

    def ext_rows(x):  # (C,H,W) -> (C,EXTR,W) with zero boundary row
        out = np.zeros((C, EXTR, W), x.dtype)
        lo, hi = r0 - 1, r0 + RH + 1
        slo, dlo = max(lo, 0), max(lo, 0) - lo
        shi = min(hi, H)
        out[:, dlo:dlo + shi - slo] = x[:, slo:shi]
        return out

    fx = inputs["front_x"][b].reshape(2, 128, HWF)
    fxh, fxl = _hilo(fx)
    cxe = ext_rows(inputs["cross_x"][b]).reshape(2, 128, EXTQ)
    cxh, cxl = _hilo(cxe)
    xh16 = inputs["front_x_hat"][b].reshape(2, 128, HWF).astype(f16)
    # catf: fp16 front_x ext rows in zero-padded [2,128,EXTR*WP] layout
    catf = np.zeros((C, EXTR, WP), f16)
    catf[:, :, 1:W + 1] = ext_rows(inputs["front_x"][b]).astype(f16)
    catf = catf.reshape(2, 128, CATW)

    valid = np.ones((EXTR, W), f32)
    if r0 == 0:
        valid[0] = 0.0
    if r0 + RH == H:
        valid[-1] = 0.0
    vq = valid.reshape(EXTQ)
    mask = np.empty((128, NBLK), f32)
    for blk in range(NBLK):
        mask[:, blk] = vq[blk * 128:(blk + 1) * 128]
    amask = (1.0 - mask) * HWF

    m = {
        "cxh": np.ascontiguousarray(cxh), "cxl": np.ascontiguousarray(cxl),
        "fxh": np.ascontiguousarray(fxh), "fxl": np.ascontiguousarray(fxl),
        "xh16": np.ascontiguousarray(xh16), "catf": np.ascontiguousarray(catf),
        "mask": mask, "amask": amask,
    }
    m.update(shared)
    return m


LAST_RES = None


def kernel(_trace=False, **inputs):
    global LAST_RES
    inputs = {k: np.asarray(v, dtype=np.float32) for k, v in inputs.items()}
    has_bqk = bool(np.any(inputs["bq"]) or np.any(inputs["bk"]))
    has_bv = bool(np.any(inputs["bv"]))
    nc = _build(has_bqk, has_bv)
    shared = _prep_shared(inputs)
    in_maps = [_prep_core_inputs(inputs, shared, core) for core in range(8)]
    kw = {"trace": True} if _trace else {}
    res = bass_utils.run_bass_kernel_spmd(nc, in_maps, core_ids=list(range(8)), **kw)
    LAST_RES = res
    out = np.empty((B, C, H, W), np.float32)
    for core in range(8):
        b, half = core // 2, core % 2
        o = res.results[core]["out"].reshape(C, RH, W)
        out[b, :, half * RH:(half + 1) * RH, :] = o
    return out


if __name__ == "__main__":
    rng = np.random.default_rng(0)
    ins = {
        "front_x": rng.standard_normal((B, C, H, W)).astype(np.float32),
        "cross_x": rng.standard_normal((B, C, H, W)).astype(np.float32),
        "front_x_hat": rng.standard_normal((B, C, H, W)).astype(np.float32),
        "Wq": (rng.standard_normal((C8, C)) / 16).astype(np.float32),
        "bq": np.zeros((C8,), np.float32),
        "Wk": (rng.standard_normal((C8, C)) / 16).astype(np.float32),
        "bk": np.zeros((C8,), np.float32),
        "Wv": (rng.standard_normal((C, C)) / 16).astype(np.float32),
        "bv": np.zeros((C,), np.float32),
        "Wf": (rng.standard_normal((C, 2 * C, 3, 3)) / 68).astype(np.float32),
        "bf": np.zeros((C,), np.float32),
    }
    out = kernel(**ins)
    print("kernel ran, out shape", out.shape, "std", out.std())


# revision 16
# speedup vs baseline: 1.3916x; 1.3916x over previous
"""CrossViewTransformer Trainium2 kernel (v3).

Shards batch B=4 x row-halves over 8 NeuronCores (pure data parallel,
one program, per-core data). Per core:
  q = Wq @ cross_ext          (32, 2176)   fp16 hi/lo split MMs (exact-ish)
  k = Wk @ front_x            (32, 4096)
  energy[j,i] = <q_j, k_i>    K=128 fp16 [qh;ql;qh;ql]x[kh;kh;kl;kl] MMs
  argmax: per-chunk DVE reduce_max + eq*iota accum, block combine
  v = Wv @ x_hat              fp16 MM -> ob-interleaved vbuf; single d=2
                              ap_gather per query-half (overlapped w/ loop)
  conv3x3([front_x; T]) * S + front_x   fp16 MMs; front half overlapped
                                        with the energy loop, T half after.

All weights pre-transposed fp16 on the host; activations pre-split fp16
hi/lo on the host. No PE transposes, no gpsimd iota/broadcast.
"""
import sys

sys.path.insert(0, "/opt/trn_rl_repo")
import numpy as np  # noqa: E402
import concourse.bacc as bacc  # noqa: E402
import concourse.mybir as mybir  # noqa: E402
import concourse.tile as tile  # noqa: E402
from concourse import bass_utils  # noqa: E402
from concourse.bass import AP  # noqa: E402

dt = mybir.dt
ALU = mybir.AluOpType
AX = mybir.AxisListType

B, C, H, W = 4, 256, 64, 64
C8 = C // 8            # 32
HWF = H * W            # 4096 keys
RH = H // 2            # 32 out rows per core
EXTR = RH + 2          # 34 ext rows (1 halo/zero row each side)
EXTQ = EXTR * W        # 2176 ext queries
NBLK = EXTQ // 128     # 17 query blocks
OUTP = RH * W          # 2048 out positions
WP = W + 2             # 66 padded width
CATW = EXTR * WP       # 2244 padded cat row-major size
NCHUNK = 4             # energy chunks of 1024 keys
VCOLS = HWF + 4        # v buffer cols (idx HWF -> zero column), 4-aligned
BSPA = 9               # blocks in gather half A (rows 0..17)
QA = BSPA * 128        # 1152 queries in half A

_CACHED = {}


def _build(has_bqk: bool, has_bv: bool):
    key = (has_bqk, has_bv)
    if key in _CACHED:
        return _CACHED[key]
    nc = bacc.Bacc("TRN2", debug=False)

    # fp16 pre-split activations
    cxh_d = nc.dram_tensor("cxh", (2, 128, EXTQ), dt.float16, kind="ExternalInput")
    cxl_d = nc.dram_tensor("cxl", (2, 128, EXTQ), dt.float16, kind="ExternalInput")
    fxh_d = nc.dram_tensor("fxh", (2, 128, HWF), dt.float16, kind="ExternalInput")
    fxl_d = nc.dram_tensor("fxl", (2, 128, HWF), dt.float16, kind="ExternalInput")
    xh16_d = nc.dram_tensor("xh16", (2, 128, HWF), dt.float16, kind="ExternalInput")
    catf_d = nc.dram_tensor("catf", (2, 128, CATW), dt.float16, kind="ExternalInput")
    # fp16 pre-transposed weights
    wqkT_d = nc.dram_tensor("wqkT", (128, 8 * C8), dt.float16, kind="ExternalInput")
    wvT_d = nc.dram_tensor("wvT", (128, 4 * 128), dt.float16, kind="ExternalInput")
    wfT_d = nc.dram_tensor("wfT", (128, 72 * 128), dt.float16, kind="ExternalInput")
    iota_d = nc.dram_tensor("iota16", (128, HWF), dt.int16, kind="ExternalInput")
    bq_d = nc.dram_tensor("bq", (C8, 1), dt.float32, kind="ExternalInput")
    bk_d = nc.dram_tensor("bk", (C8, 1), dt.float32, kind="ExternalInput")
    bv_d = nc.dram_tensor("bv", (128, 2), dt.float32, kind="ExternalInput")
    bf_d = nc.dram_tensor("bf", (128, 2), dt.float32, kind="ExternalInput")
    mask_d = nc.dram_tensor("mask", (128, NBLK), dt.float32, kind="ExternalInput")
    amask_d = nc.dram_tensor("amask", (128, NBLK), dt.float32, kind="ExternalInput")

    out_d = nc.dram_tensor("out", (2, 128, OUTP), dt.float32, kind="ExternalOutput")
    dbg_arg_d = nc.dram_tensor("dbg_arg", (128, NBLK), dt.float32, kind="ExternalOutput")
    dbg_s_d = nc.dram_tensor("dbg_s", (128, NBLK), dt.float32, kind="ExternalOutput")

    with tile.TileContext(nc) as tc:
        _body(nc, tc, locals(), has_bqk, has_bv)
    nc.compile()
    _CACHED[key] = nc
    return nc


def _body(nc, tc, T, has_bqk, has_bv):
    F32, F16, I16 = dt.float32, dt.float16, dt.int16

    with tc.tile_pool(name="dramscr", bufs=1, space="DRAM") as DR, \
         tc.tile_pool(name="persist", bufs=1) as P, \
         tc.tile_pool(name="pse", bufs=3, space="PSUM") as PSE, \
         tc.tile_pool(name="psb", bufs=2, space="PSUM") as PSB:

        # ---------- persistent tiles ----------
        wqkT = P.tile([128, 8 * C8], F16, tag="wqkT")
        wvT = P.tile([128, 4 * 128], F16, tag="wvT")
        wfT = P.tile([128, 72 * 128], F16, tag="wfT")
        qstack = P.tile([128, EXTQ], F16, tag="qstack")
        kstack = P.tile([128, HWF], F16, tag="kstack")
        vbuf01 = P.tile([128, VCOLS, 2], F32, tag="vbuf01")
        iota16 = P.tile([128, HWF], I16, tag="iota16")
        scr = P.tile([128, 1024], F16, tag="scr")
        cats = []
        for i in range(4):
            ct = P.tile([128, CATW], F16, tag=f"cat{i}")
            cats.append(ct)
        convacc = P.tile([128, 8, 512], F32, tag="convacc")  # (ob*4+g)
        s128 = P.tile([128, OUTP], F32, tag="s128")
        SM = P.tile([128, 128], F32, tag="smalls")
        Af = SM[:, 0:NBLK]
        Mg = SM[:, 17:17 + NBLK]
        arg2 = SM[:, 34:34 + NBLK]
        maskt = SM[:, 51:51 + NBLK]
        amaskt = SM[:, 68:68 + NBLK]
        bqs = SM[0:C8, 85:87]
        bvs = SM[:, 87:89]
        bfs = SM[:, 89:91]
        mch = SM[:, 91:95]
        ach = SM[:, 95:99]
        sel = SM[:, 99:103]
        af16 = P.tile([128, NBLK], I16, tag="af16")
        idxw = P.tile([128, EXTQ // 16], I16, tag="idxw")

        # ---------- input DMAs ----------
        nc.sync.dma_start(wqkT[:, :], T["wqkT_d"].ap())

        ACT_cm = tc.tile_pool(name="actpool", bufs=1)
        ACT = ACT_cm.__enter__()
        cxh = ACT.tile([128, 2, EXTQ], F16, tag="cxh")
        cxl = ACT.tile([128, 2, EXTQ], F16, tag="cxl")
        fxh = ACT.tile([128, 2, HWF], F16, tag="fxh")
        fxl = ACT.tile([128, 2, HWF], F16, tag="fxl")
        xh16 = ACT.tile([128, 2, HWF], F16, tag="xh16")
        for cb in range(2):
            nc.sync.dma_start(cxh[:, cb], T["cxh_d"].ap()[cb])
            nc.sync.dma_start(cxl[:, cb], T["cxl_d"].ap()[cb])
        for cb in range(2):
            nc.sync.dma_start(fxh[:, cb], T["fxh_d"].ap()[cb])
            nc.sync.dma_start(fxl[:, cb], T["fxl_d"].ap()[cb])
        nc.sync.dma_start(wvT[:, :], T["wvT_d"].ap())
        for cb in range(2):
            nc.sync.dma_start(xh16[:, cb], T["xh16_d"].ap()[cb])
        nc.sync.dma_start(wfT[:, :], T["wfT_d"].ap())
        for cb in range(2):
            nc.sync.dma_start(cats[cb][:, :], T["catf_d"].ap()[cb])
        nc.sync.dma_start(iota16[:, :], T["iota_d"].ap())
        nc.sync.dma_start(maskt[:, :], T["mask_d"].ap())
        nc.sync.dma_start(amaskt[:, :], T["amask_d"].ap())
        if has_bqk:
            nc.sync.dma_start(bqs[:, 0:1], T["bq_d"].ap())
            nc.sync.dma_start(bqs[:, 1:2], T["bk_d"].ap())
        if has_bv:
            nc.sync.dma_start(bvs[:, :], T["bv_d"].ap())
        nc.sync.dma_start(bfs[:, :], T["bf_d"].ap())

        nc.vector.memset(cats[2][:, :], 0.0)
        nc.vector.memset(cats[3][:, :], 0.0)
        nc.vector.memset(vbuf01[:, HWF:VCOLS, :], 0.0)

        # ---------- q, k (fp16 hi/lo x hi/lo accumulation) ----------
        def qk_mm(which, xh_t, xl_t, npos, stack, hrows, lrows):
            nchunks = (npos + 511) // 512
            for ch in range(nchunks):
                n0, n1 = ch * 512, min((ch + 1) * 512, npos)
                pq = PSB.tile([C8, 512], F32, tag="ps512")
                first = True
                for cb in range(2):
                    for wsplit in range(2):
                        wcol = ((which * 2 + cb) * 2 + wsplit) * C8
                        for xs, xt in ((0, xh_t), (1, xl_t)):
                            nc.tensor.matmul(
                                pq[:, 0:n1 - n0],
                                wqkT[:, wcol:wcol + C8],
                                xt[:, cb, n0:n1],
                                start=first, stop=(cb == 1 and wsplit == 1 and xs == 1))
                            first = False
                hi = stack[hrows[0]:hrows[0] + C8, n0:n1]
                if has_bqk:
                    nc.vector.tensor_scalar(
                        out=hi, in0=pq[:, 0:n1 - n0],
                        scalar1=bqs[:, which:which + 1], scalar2=None, op0=ALU.add)
                else:
                    nc.scalar.copy(hi, pq[:, 0:n1 - n0])
                nc.vector.scalar_tensor_tensor(
                    stack[lrows[0]:lrows[0] + C8, n0:n1], pq[:, 0:n1 - n0],
                    bqs[:, which:which + 1] if has_bqk else 0.0, hi,
                    op0=ALU.add, op1=ALU.subtract)
                for extra in hrows[1:]:
                    nc.vector.tensor_copy(stack[extra:extra + C8, n0:n1], hi)
                for extra in lrows[1:]:
                    nc.vector.tensor_copy(stack[extra:extra + C8, n0:n1],
                                          stack[lrows[0]:lrows[0] + C8, n0:n1])

        # qstack rows: [qh, ql, qh, ql] ; kstack rows: [kh, kh, kl, kl]
        qk_mm(0, cxh, cxl, EXTQ, qstack, hrows=(0, 64), lrows=(32, 96))
        qk_mm(1, fxh, fxl, HWF, kstack, hrows=(0, 32), lrows=(64, 96))

        # ---------- filler emitters (PE work overlapped with energy DVE) ----------
        def emit_v_chunk(ch):
            for ob in range(2):
                pv = PSB.tile([128, 512], F32, tag="ps512")
                for cb in range(2):
                    nc.tensor.matmul(pv[:, :],
                                     wvT[:, (cb * 2 + ob) * 128:(cb * 2 + ob + 1) * 128],
                                     xh16[:, cb, ch * 512:(ch + 1) * 512],
                                     start=(cb == 0), stop=(cb == 1))
                vdst = vbuf01[:, ch * 512:(ch + 1) * 512, ob:ob + 1]
                vdst = vdst.rearrange("p n one -> p (n one)")
                if has_bv:
                    nc.vector.tensor_scalar(
                        out=vdst, in0=pv[:, :],
                        scalar1=bvs[:, ob:ob + 1], scalar2=None, op0=ALU.add)
                else:
                    nc.scalar.copy(vdst, pv[:, :])

        def emit_conv_half(ob, g, cb4s, pc):
            for cb4 in cb4s:
                for tap in range(9):
                    dy, dx = tap // 3, tap % 3
                    col = ((cb4 * 9 + tap) * 2 + ob) * 128
                    catv = cats[cb4][:, :].rearrange("p (r wp) -> p r wp", wp=WP)
                    rhs = catv[:, g * 8 + dy:g * 8 + dy + 8, dx:dx + W]
                    nc.tensor.matmul(pc[:, :], wfT[:, col:col + 128], rhs,
                                     start=(cb4 == cb4s[0] and tap == 0),
                                     stop=(cb4 == cb4s[-1] and tap == 8))

        def emit_front_group(ob, g):
            pc = PSB.tile([128, 512], F32, tag="ps512")
            emit_conv_half(ob, g, (0, 1), pc)
            nc.scalar.copy(convacc[:, ob * 4 + g, :], pc[:, :])

        fillers = []
        for ch in range(8):
            fillers.append(lambda ch=ch: emit_v_chunk(ch))
        for ob in range(2):
            for g in range(4):
                fillers.append(lambda ob=ob, g=g: emit_front_group(ob, g))

        # ---------- gather-half plumbing ----------
        wrapA_t = DR.tile([QA], I16, tag="wrapA")
        wrapB_t = DR.tile([EXTQ - QA], I16, tag="wrapB")
        srow_t = DR.tile([EXTQ], F32, tag="srowd")
        TGbox = {}

        def emit_arg_relayout(b0, b1, wtile):
            # masked arg, int16 cast, wrap DMA, idxw load for blocks [b0, b1)
            nb = b1 - b0
            nc.vector.tensor_tensor(arg2[:, b0:b1], Af[:, b0:b1],
                                    maskt[:, b0:b1], op=ALU.mult)
            nc.vector.tensor_tensor(arg2[:, b0:b1], arg2[:, b0:b1],
                                    amaskt[:, b0:b1], op=ALU.add)
            nc.vector.tensor_copy(af16[:, b0:b1], arg2[:, b0:b1])
            wsrc = wtile[:]
            wdst = AP(wsrc.tensor, wsrc.offset, [[1, 8], [nb * 8, 16], [8, nb]])
            nc.sync.dma_start(wdst, af16[:, b0:b1])
            wview = wtile[:].rearrange("(p0 s) -> p0 s", p0=16)
            for g in range(8):
                nc.sync.dma_start(idxw[16 * g:16 * (g + 1), b0 * 8:b1 * 8],
                                  wview[:, :])

        def emit_gather(q0, q1):
            tg = TGbox["tg"]
            nc.gpsimd.ap_gather(tg[:, q0:q1, :], vbuf01[:, :, :],
                                idxw[:, q0 // 16:q1 // 16],
                                channels=128, num_elems=VCOLS, d=2, num_idxs=q1 - q0)

        def emit_catcopy(q0, q1):
            tg = TGbox["tg"]
            r0, r1 = q0 // W, q1 // W
            for ob in range(2):
                src = tg[:, q0:q1, ob:ob + 1].rearrange("p q one -> p (q one)")
                src = src.rearrange("p (r w) -> p r w", w=W)
                dst = cats[2 + ob][:, :].rearrange(
                    "p (r wp) -> p r wp", wp=WP)[:, r0:r1, 1:W + 1]
                nc.scalar.copy(dst, src)

        # ---------- energy + argmax ----------
        for b in range(NBLK):
            for c in range(NCHUNK):
                pe = PSE.tile([128, 1024], F32, tag="pe")
                nc.tensor.matmul(pe[:, 0:512], qstack[:, b * 128:(b + 1) * 128],
                                 kstack[:, c * 1024:c * 1024 + 512], start=True, stop=True)
                nc.tensor.matmul(pe[:, 512:1024], qstack[:, b * 128:(b + 1) * 128],
                                 kstack[:, c * 1024 + 512:(c + 1) * 1024], start=True, stop=True)
                nc.vector.tensor_reduce(mch[:, c:c + 1], pe[:, :], axis=AX.X, op=ALU.max)
                nc.vector.scalar_tensor_tensor(
                    scr[:, :], pe[:, :], mch[:, c:c + 1],
                    iota16[:, c * 1024:(c + 1) * 1024],
                    op0=ALU.is_equal, op1=ALU.mult, accum_out=ach[:, c:c + 1])
            nc.vector.tensor_reduce(Mg[:, b:b + 1], mch[:, :], axis=AX.X, op=ALU.max)
            nc.vector.scalar_tensor_tensor(
                sel[:, :], mch[:, :], Mg[:, b:b + 1], ach[:, :],
                op0=ALU.is_equal, op1=ALU.mult, accum_out=Af[:, b:b + 1])
            # one PE filler batch per block
            if b < len(fillers):
                fillers[b]()
            if b == 7:
                ACT_cm.__exit__(None, None, None)  # cx/fx/xh dead: free SBUF
            if b == 8:
                TG_cm = tc.tile_pool(name="tgpool", bufs=1)
                TG = TG_cm.__enter__()
                tgtile = TG.tile([128, EXTQ, 2], F32, tag="tg")
                TGbox["tg"] = tgtile
                emit_arg_relayout(0, BSPA, wrapA_t)
                emit_gather(0, QA)
            if b == 13:
                emit_catcopy(0, QA)
        for f in fillers[NBLK:]:
            f()

        nc.sync.dma_start(T["dbg_s_d"].ap(), Mg[:, :])
        nc.sync.dma_start(T["dbg_arg_d"].ap(), Af[:, :])

        emit_arg_relayout(BSPA, NBLK, wrapB_t)
        emit_gather(QA, EXTQ)

        # S row: srow[q=b*128+p] = Mg[p,b]; then stride-0 broadcast of out cols
        ssrc = srow_t[:]
        sdst = AP(ssrc.tensor, ssrc.offset, [[1, 128], [128, NBLK]])
        nc.sync.dma_start(sdst, Mg[:, :])
        sbc = AP(ssrc.tensor, ssrc.offset + W, [[0, 128], [1, OUTP]])
        nc.sync.dma_start(s128[:, :], sbc)

        emit_catcopy(QA, EXTQ)

        # ---------- conv T-half + assembly ----------
        S_cm = tc.tile_pool(name="stream", bufs=2)
        S = S_cm.__enter__()
        for g in range(4):
            for ob in range(2):
                pc = PSB.tile([128, 512], F32, tag="ps512")
                emit_conv_half(ob, g, (2, 3), pc)
                stage = S.tile([128, 512], F32, tag="stage")
                nc.vector.scalar_tensor_tensor(
                    stage[:, :], pc[:, :], bfs[:, ob:ob + 1],
                    convacc[:, ob * 4 + g, :], op0=ALU.add, op1=ALU.add)
                nc.vector.tensor_tensor(stage[:, :], stage[:, :],
                                        s128[:, g * 512:(g + 1) * 512], op=ALU.mult)
                fcatv = cats[ob][:, :].rearrange("p (r wp) -> p r wp", wp=WP)
                front_mid = fcatv[:, g * 8 + 1:g * 8 + 9, 1:1 + W]
                nc.vector.tensor_tensor(stage[:, :], stage[:, :], front_mid, op=ALU.add)
                nc.sync.dma_start(T["out_d"].ap()[ob][:, g * 512:(g + 1) * 512],
                                  stage[:, :])
        S_cm.__exit__(None, None, None)
        TG_cm.__exit__(None, None, None)


def _prep_shared(inputs):
    """Weight prep shared by all cores: pre-transposed fp16 (+hi/lo for q/k)."""
    f16, f32 = np.float16, np.float32
    Wq, Wk, Wv = inputs["Wq"], inputs["Wk"], inputs["Wv"]
    Wf = inputs["Wf"].reshape(C, 2 * C, 9)

    wqkT = np.zeros((128, 8 * C8), f16)
    for which, Wx in ((0, Wq), (1, Wk)):
        for cb in range(2):
            blk = np.ascontiguousarray(Wx[:, cb * 128:(cb + 1) * 128].T)  # [128, 32] f32
            hi = blk.astype(f16)
            lo = (blk - hi.astype(f32)).astype(f16)
            base = ((which * 2 + cb) * 2) * C8
            wqkT[:, base:base + C8] = hi
            wqkT[:, base + C8:base + 2 * C8] = lo

    wvT = np.zeros((128, 4 * 128), f16)
    for ob in range(2):
        for cb in range(2):
            wvT[:, (cb * 2 + ob) * 128:(cb * 2 + ob + 1) * 128] = \
                Wv[ob * 128:(ob + 1) * 128, cb * 128:(cb + 1) * 128].T.astype(f16)

    wfT = np.zeros((128, 72 * 128), f16)
    for ob in range(2):
        for cb4 in range(4):
            for tap in range(9):
                col = ((cb4 * 9 + tap) * 2 + ob) * 128
                wfT[:, col:col + 128] = \
                    Wf[ob * 128:(ob + 1) * 128, cb4 * 128:(cb4 + 1) * 128, tap].T.astype(f16)

    iota16 = np.broadcast_to(np.arange(HWF, dtype=np.int16), (128, HWF)).copy()

    return {
        "wqkT": wqkT, "wvT": wvT, "wfT": wfT, "iota16": iota16,
        "bq": inputs["bq"].reshape(C8, 1).astype(f32),
        "bk": inputs["bk"].reshape(C8, 1).astype(f32),
        "bv": np.ascontiguousarray(inputs["bv"].reshape(2, 128).T).astype(f32),
        "bf": np.ascontiguousarray(inputs["bf"].reshape(2, 128).T).astype(f32),
    }


def _hilo(x):
    f16, f32 = np.float16, np.float32
    hi = x.astype(f16)
    lo = (x - hi.astype(f32)).astype(f16)
    return hi, lo


def _prep_core_inputs(inputs, shared, core):
    f16, f32 = np.float16, np.float32
    b, half = core // 2, core % 2
    r0 = half * RH

    def ext_rows(x):  # (C,H,W) -> (C,EXTR,W) with zero boundary row
        out = np.zeros((C, EXTR, W), x.dtype)
        lo, hi = r0 - 1, r0 + RH + 1
        slo, dlo = max(lo, 0), max(lo, 0) - lo
        shi = min(hi, H)
        out[:, dlo:dlo + shi - slo] = x[:, slo:shi]
        return out

    fx = inputs["front_x"][b].reshape(2, 128, HWF)
    fxh, fxl = _hilo(fx)
    cxe = ext_rows(inputs["cross_x"][b]).reshape(2, 128, EXTQ)
    cxh, cxl = _hilo(cxe)
    xh16 = inputs["front_x_hat"][b].reshape(2, 128, HWF).astype(f16)
    catf = np.zeros((C, EXTR, WP), f16)
    catf[:, :, 1:W + 1] = ext_rows(inputs["front_x"][b]).astype(f16)
    catf = catf.reshape(2, 128, CATW)

    valid = np.ones((EXTR, W), f32)
    if r0 == 0:
        valid[0] = 0.0
    if r0 + RH == H:
        valid[-1] = 0.0
    vq = valid.reshape(EXTQ)
    mask = np.empty((128, NBLK), f32)
    for blk in range(NBLK):
        mask[:, blk] = vq[blk * 128:(blk + 1) * 128]
    amask = (1.0 - mask) * HWF

    m = {
        "cxh": np.ascontiguousarray(cxh), "cxl": np.ascontiguousarray(cxl),
        "fxh": np.ascontiguousarray(fxh), "fxl": np.ascontiguousarray(fxl),
        "xh16": np.ascontiguousarray(xh16), "catf": np.ascontiguousarray(catf),
        "mask": mask, "amask": amask,
    }
    m.update(shared)
    return m


LAST_RES = None


def kernel(_trace=False, **inputs):
    global LAST_RES
    inputs = {k: np.asarray(v, dtype=np.float32) for k, v in inputs.items()}
    has_bqk = bool(np.any(inputs["bq"]) or np.any(inputs["bk"]))
    has_bv = bool(np.any(inputs["bv"]))
    nc = _build(has_bqk, has_bv)
    shared = _prep_shared(inputs)
    in_maps = [_prep_core_inputs(inputs, shared, core) for core in range(8)]
    kw = {"trace": True} if _trace else {}
    res = bass_utils.run_bass_kernel_spmd(nc, in_maps, core_ids=list(range(8)), **kw)
    LAST_RES = res
    out = np.empty((B, C, H, W), np.float32)
    for core in range(8):
        b, half = core // 2, core % 2
        o = res.results[core]["out"].reshape(C, RH, W)
        out[b, :, half * RH:(half + 1) * RH, :] = o
    return out


if __name__ == "__main__":
    rng = np.random.default_rng(0)
    ins = {
        "front_x": rng.standard_normal((B, C, H, W)).astype(np.float32),
        "cross_x": rng.standard_normal((B, C, H, W)).astype(np.float32),
        "front_x_hat": rng.standard_normal((B, C, H, W)).astype(np.float32),
        "Wq": (rng.standard_normal((C8, C)) / 16).astype(np.float32),
        "bq": np.zeros((C8,), np.float32),
        "Wk": (rng.standard_normal((C8, C)) / 16).astype(np.float32),
        "bk": np.zeros((C8,), np.float32),
        "Wv": (rng.standard_normal((C, C)) / 16).astype(np.float32),
        "bv": np.zeros((C,), np.float32),
        "Wf": (rng.standard_normal((C, 2 * C, 3, 3)) / 68).astype(np.float32),
        "bf": np.zeros((C,), np.float32),
    }
    out = kernel(**ins)
    print("kernel ran, out shape", out.shape, "std", out.std())
